# revision 17
# baseline (speedup 1.0000x reference)
"""Cross-attention Trainium2 kernel (nn_CrossAttention, B=2, L=2048, D=1024,
Dctx=768, 16 heads x 64).

Sharding: 8 cores = 2 (batch) x 4 (head-groups of 4 heads). Each core computes
its batch's Q/K/V projections for its 4 heads, flash-style attention in the
transposed (S^T) domain, and a partial output projection; the host sums the
head-group partials and adds b_o.

All activations live transposed on-chip (xT, ctxT, qT, kT, attnT) so every
matmul contracts over the partition dim with no on-chip transposes; operands
are fp16 (full PE streaming rate) with fp32 PSUM accumulation. Heads are
processed in pairs: the pair's scores matmuls contract K=64 on PE row-groups
(0,0) and (64,0) and stream CONCURRENTLY into the two banks of one [128,1024]
PSUM tile, so a head-pair's scores cost one stream instead of two. One
1024-wide exp covers both heads. V tiles are padded to 128 columns (64 v + 32
ones for the softmax denominator + 32 zeros) so every stationary load takes
the fast-weight-load path. Output partials are fp16, summed on the host.
"""
import numpy as np

import concourse.bass as bass
import concourse.tile as tile
from concourse import bacc, mybir, bass_utils

F16 = mybir.dt.float16
F32 = mybir.dt.float32
EXP = mybir.ActivationFunctionType.Exp
IDENT = mybir.ActivationFunctionType.Identity

# Problem shape (hardcoded per harness contract)
B, LQ, D = 2, 2048, 1024
DCTX = 768
NH, HD = 16, 64
SCALE = 1.0 / 8.0  # 1/sqrt(64)

# Per-core shard: 4 heads (one group), one batch
GH = 4                # heads per core
ONES = 32             # d-replication rows per head
VW = 128              # per-head v_t width: 64 v + 32 ones + 32 zero pad (FWL)
VAW = GH * VW         # 512
GD = GH * HD          # 256: real v columns per chunk
KT_Q = D // 128       # 8
KT_C = DCTX // 128    # 6
NLK = LQ // 128       # 16 key tiles
NS = LQ // 512        # 4 query 512-slices
HALF = 1024


def _build():
    nc = bacc.Bacc("TRN2", target_bir_lowering=False, debug=False,
                   enable_asserts=False, num_devices=8)

    xT_d = nc.dram_tensor("xT", (D, LQ), F16, kind="ExternalInput").ap()
    cT_d = nc.dram_tensor("ctxT", (DCTX, LQ), F16, kind="ExternalInput").ap()
    wq_d = nc.dram_tensor("wq", (D, 256), F16, kind="ExternalInput").ap()
    wk_d = nc.dram_tensor("wk", (DCTX, 256), F16, kind="ExternalInput").ap()
    wv_d = nc.dram_tensor("wv", (DCTX, GD), F16, kind="ExternalInput").ap()
    wo_d = nc.dram_tensor("wo", (256, D), F16, kind="ExternalInput").ap()
    bq_d = nc.dram_tensor("bq", (128, 2), F32, kind="ExternalInput").ap()
    bk_d = nc.dram_tensor("bk", (128, 2), F32, kind="ExternalInput").ap()
    bvb_d = nc.dram_tensor("bvb", (128, GD), F32, kind="ExternalInput").ap()
    out_d = nc.dram_tensor("outT", (D, LQ), F16, kind="ExternalOutput").ap()

    with tile.TileContext(nc) as tc:
        with tc.tile_pool(name="w", bufs=1) as wp, \
             tc.tile_pool(name="xt", bufs=10) as xtp, \
             tc.tile_pool(name="ct", bufs=24) as ctp, \
             tc.tile_pool(name="act", bufs=1) as actp, \
             tc.tile_pool(name="expp", bufs=3) as expp, \
             tc.tile_pool(name="scrp", bufs=3) as scrp, \
             tc.tile_pool(name="rdp", bufs=3) as rdp, \
             tc.tile_pool(name="outp", bufs=3) as outp, \
             tc.tile_pool(name="ps_w", bufs=2, space="PSUM") as ps_w, \
             tc.tile_pool(name="ps_s", bufs=2, space="PSUM") as ps_s, \
             tc.tile_pool(name="ps_w2", bufs=1, space="PSUM") as ps_w2, \
             tc.tile_pool(name="ps_o", bufs=1, space="PSUM") as ps_o:

            # ---- weight/bias tiles (DMAs issued interleaved below) ----
            wq_t = wp.tile([128, KT_Q * 256], F16, tag="wq")
            wk_t = wp.tile([128, KT_C * 256], F16, tag="wk")
            wv_t = wp.tile([128, KT_C * GD], F16, tag="wv")
            wo_t = wp.tile([128, 2 * D], F16, tag="wo")
            bq_t = wp.tile([128, 2], F32, tag="bq")
            bk_t = wp.tile([128, 2], F32, tag="bk")
            bvb_t = wp.tile([128, GD], F32, tag="bvb")

            # K proj needs these first
            nc.sync.dma_start(wk_t[:].rearrange("p (kt m) -> p kt m", m=256),
                              wk_d.rearrange("(kt p) m -> p kt m", p=128))
            nc.sync.dma_start(bk_t[:], bk_d[:])

            # ---- persistent activation tiles ----
            qT = [actp.tile([128, LQ], F16, tag=f"qT{p}", name=f"qT{p}")
                  for p in range(2)]
            kT = [actp.tile([128, LQ], F16, tag=f"kT{p}", name=f"kT{p}")
                  for p in range(2)]
            v_t = actp.tile([128, NLK * VAW], F16, tag="v")
            aT = [actp.tile([128, LQ], F16, tag=f"aT{p}", name=f"aT{p}")
                  for p in range(2)]

            # constant ones (softmax denominator) / zero-pad rows of v_t
            v4 = v_t[:].rearrange("p (j w) -> p j w", w=VAW)
            for h in range(GH):
                nc.vector.memset(v4[:, :, VW * h + HD:VW * h + HD + ONES], 1.0)
                nc.vector.memset(v4[:, :, VW * h + HD + ONES:VW * (h + 1)], 0.0)

            # ---- K+V projections interleaved per ctx 512-slice ----
            ct_tiles = {}
            for s in range(NS):
                for kt in range(KT_C):
                    t = ctp.tile([128, 512], F16, tag="ct")
                    nc.sync.dma_start(
                        t[:], cT_d[128 * kt:128 * (kt + 1), 512 * s:512 * (s + 1)])
                    ct_tiles[(kt, s)] = t
                # stagger the remaining input loads behind the ct slices
                if s == 0:
                    nc.sync.dma_start(
                        wv_t[:].rearrange("p (kt m) -> p kt m", m=GD),
                        wv_d.rearrange("(kt p) m -> p kt m", p=128))
                    nc.sync.dma_start(bvb_t[:], bvb_d[:])
                    nc.sync.dma_start(
                        wq_t[:].rearrange("p (kt m) -> p kt m", m=256),
                        wq_d.rearrange("(kt p) m -> p kt m", p=128))
                    nc.sync.dma_start(bq_t[:], bq_d[:])
                elif s == 1:
                    nc.sync.dma_start(
                        wo_t[:].rearrange("p (p2 m) -> p p2 m", m=1024),
                        wo_d.rearrange("(p2 p) m -> p p2 m", p=128))
                for p in range(2):
                    ps = ps_w.tile([128, 512], F32, tag="mm")
                    for kt in range(KT_C):
                        nc.tensor.matmul(
                            ps[:], wk_t[:, 256 * kt + 128 * p:256 * kt + 128 * (p + 1)],
                            ct_tiles[(kt, s)][:],
                            start=(kt == 0), stop=(kt == KT_C - 1))
                    nc.scalar.activation(
                        kT[p][:, 512 * s:512 * (s + 1)], ps[:], IDENT,
                        bias=bk_t[:, p:p + 1])
                for jj in range(4):
                    j = 4 * s + jj
                    ps = ps_w.tile([128, 512], F32, tag="mm")
                    for kt in range(KT_C):
                        nc.tensor.matmul(
                            ps[:, 0:GD],
                            ct_tiles[(kt, s)][:, 128 * jj:128 * (jj + 1)],
                            wv_t[:, GD * kt:GD * (kt + 1)],
                            start=(kt == 0), stop=(kt == KT_C - 1))
                    for h in range(GH):
                        nc.vector.tensor_add(
                            v_t[:, VAW * j + VW * h:VAW * j + VW * h + HD],
                            ps[:, HD * h:HD * (h + 1)],
                            bvb_t[:, HD * h:HD * (h + 1)])

            # ---- Q projection; slice 0 up front, rest interleaved into
            # attention (q-slice si+1 computed during attention on si)
            def q_proj(s, pp):
                xt_tiles = []
                for kt in range(KT_Q):
                    t = xtp.tile([128, 512], F16, tag="xt",
                                 name=f"xt{s}_{pp}_{kt}")
                    nc.sync.dma_start(
                        t[:], xT_d[128 * kt:128 * (kt + 1), 512 * s:512 * (s + 1)])
                    xt_tiles.append(t)
                ps = ps_o.tile([128, 512], F32, tag="o", name=f"qps{s}_{pp}")
                for kt in range(KT_Q):
                    nc.tensor.matmul(
                        ps[:], wq_t[:, 256 * kt + 128 * pp:256 * kt + 128 * (pp + 1)],
                        xt_tiles[kt][:],
                        start=(kt == 0), stop=(kt == KT_Q - 1))
                qsc = scrp.tile([128, 512], F32, tag="scr", name=f"qsc{s}_{pp}")
                nc.vector.tensor_scalar_add(qsc[:], ps[:], bq_t[:, pp:pp + 1])
                nc.vector.tensor_copy(qT[pp][:, 512 * s:512 * (s + 1)], qsc[:])

            for pp in range(2):
                q_proj(0, pp)

            def out_proj_group(mo, s, pool=None, on_act=False):
                pool = pool if pool is not None else ps_o
                tg = "o" if pool is ps_o else ("mm" if pool is ps_w else "mm2")
                ops = pool.tile([128, 512], F32, tag=tg, name=f"ops{mo}_{s}")
                for p in range(2):
                    nc.tensor.matmul(
                        ops[:], wo_t[:, D * p + 128 * mo:D * p + 128 * (mo + 1)],
                        aT[p][:, 512 * s:512 * (s + 1)],
                        start=(p == 0), stop=(p == 1))
                ot = outp.tile([128, 512], F16, tag="out")
                if on_act:
                    nc.scalar.copy(ot[:], ops[:])
                else:
                    nc.vector.tensor_copy(ot[:], ops[:])
                nc.sync.dma_start(
                    out_d[128 * mo:128 * (mo + 1), 512 * s:512 * (s + 1)], ot[:])

            # ---- attention: q-slice outer; slice s-1's output projection
            # interleaved into slice s's j-loop to keep the exp stream dense
            for si in range(NS):
                cols = slice(512 * si, 512 * (si + 1))
                for p in range(2):
                    hA, hB = 2 * p, 2 * p + 1
                    pa = {0: ps_w.tile([128, 512], F32, tag="mm",
                                       name=f"pa{si}_{p}_0"),
                          1: ps_w2.tile([128, 512], F32, tag="mm2",
                                        name=f"pa{si}_{p}_1")}
                    for j in range(NLK):
                        ks = slice(128 * j, 128 * (j + 1))
                        st = ps_s.tile([128, HALF], F32, tag="s")
                        # concurrent PE row-group pair: head A rows 0:64,
                        # head B rows 64:128, disjoint PSUM banks
                        nc.tensor.matmul(
                            st[:, 0:512], kT[p][0:64, ks], qT[p][0:64, cols],
                            start=True, stop=True)
                        nc.tensor.matmul(
                            st[:, 512:1024], kT[p][64:128, ks],
                            qT[p][64:128, cols], start=True, stop=True)
                        ex = expp.tile([128, HALF], F16, tag="expS")
                        nc.scalar.activation(ex[:], st[:], EXP, scale=SCALE)
                        nc.tensor.matmul(
                            pa[0][:],
                            v_t[:, VAW * j + VW * hA:VAW * j + VW * (hA + 1)],
                            ex[:, 0:512],
                            start=(j == 0), stop=(j == NLK - 1))
                        nc.tensor.matmul(
                            pa[1][:],
                            v_t[:, VAW * j + VW * hB:VAW * j + VW * (hB + 1)],
                            ex[:, 512:1024],
                            start=(j == 0), stop=(j == NLK - 1))
                        if si > 0 and j in (2, 6, 10, 14):
                            out_proj_group(4 * p + (j - 2) // 4, si - 1)
                        if si < NS - 1 and j == 12:
                            q_proj(si + 1, p)
                    # normalize: attnT = attnU * (1/d); d-block replicated to
                    # 64 partitions via PSUM->SBUF shifts (SBUF->SBUF illegal)
                    for hh in (1, 0):
                        r0 = 64 * hh
                        pan = pa[hh]
                        dsb = rdp.tile([64, 512], F32, tag="dsb")
                        nc.vector.tensor_copy(dsb[0:32, :], pan[64:96, :])
                        nc.vector.tensor_copy(dsb[32:64, :], pan[64:96, :])
                        rd = rdp.tile([64, 512], F32, tag="rd")
                        rds = rdp.tile([64, 512], F32, tag="rds")
                        nc.vector.reciprocal_approx_accurate(
                            rd[:], dsb[:], rds[:])
                        nc.vector.tensor_mul(
                            aT[p][r0:r0 + 64, cols], pan[0:64, :], rd[:])
            for mo in range(D // 128):
                out_proj_group(mo, NS - 1,
                               pool=(ps_o if mo % 2 == 0 else ps_w2),
                               on_act=(mo % 2 == 1))

    nc.compile()
    return nc


_NC_CACHE = []


def _get_nc():
    if not _NC_CACHE:
        _NC_CACHE.append(_build())
    return _NC_CACHE[0]


def kernel_run(inputs, trace=False, **kw):
    """Run on HW; returns (full_output, BassKernelResults)."""
    x = np.asarray(inputs["x"], np.float32)
    context = np.asarray(inputs["context"], np.float32)
    w_q = np.asarray(inputs["w_q"], np.float32)
    b_q = np.asarray(inputs["b_q"], np.float32)
    w_k = np.asarray(inputs["w_k"], np.float32)
    b_k = np.asarray(inputs["b_k"], np.float32)
    w_v = np.asarray(inputs["w_v"], np.float32)
    b_v = np.asarray(inputs["b_v"], np.float32)
    w_o = np.asarray(inputs["w_o"], np.float32)
    b_o = np.asarray(inputs["b_o"], np.float32)

    f16 = np.float16
    xT_h = [np.ascontiguousarray(x[b].T).astype(f16) for b in range(B)]
    cT_h = [np.ascontiguousarray(context[b].T).astype(f16) for b in range(B)]

    maps = []
    for c in range(8):
        b, g = c // 4, c % 4
        hs = slice(256 * g, 256 * (g + 1))
        maps.append({
            "xT": xT_h[b],
            "ctxT": cT_h[b],
            "wq": np.ascontiguousarray(w_q[:, hs]).astype(f16),
            "wk": np.ascontiguousarray(w_k[:, hs]).astype(f16),
            "wv": np.ascontiguousarray(w_v[:, hs]).astype(f16),
            "wo": np.ascontiguousarray(w_o[hs, :]).astype(f16),
            "bq": np.ascontiguousarray(b_q[hs].reshape(2, 128).T),
            "bk": np.ascontiguousarray(b_k[hs].reshape(2, 128).T),
            "bvb": np.ascontiguousarray(
                np.broadcast_to(b_v[None, hs], (128, GD)).astype(np.float32)),
        })

    nc = _get_nc()
    res = bass_utils.run_bass_kernel_spmd(nc, maps, core_ids=list(range(8)),
                                          trace=trace, **kw)
    out = np.empty((B, LQ, D), np.float32)
    for b in range(B):
        acc = res.results[4 * b]["outT"].astype(np.float32)
        for g in range(1, 4):
            acc = acc + res.results[4 * b + g]["outT"].astype(np.float32)
        out[b] = acc.T + b_o[None, :]
    return out, res


def kernel(**inputs) -> np.ndarray:
    out, _ = kernel_run(inputs)
    return out


# revision 18
# speedup vs baseline: 1.0886x; 1.0886x over previous
"""Cross-attention Trainium2 kernel (nn_CrossAttention, B=2, L=2048, D=1024,
Dctx=768, 16 heads x 64).

Sharding: 8 cores = 2 (batch) x 4 (head-groups of 4 heads). Each core computes
its batch's Q/K/V projections for its 4 heads, flash-style attention in the
transposed (S^T) domain, and a partial output projection; the host sums the
head-group partials and adds b_o.

All activations live transposed on-chip (xT, ctxT, qT, kT, attnT) so every
matmul contracts over the partition dim with no on-chip transposes; operands
are fp16 (full PE streaming rate) with fp32 PSUM accumulation. Heads are
processed in pairs: the pair's scores matmuls contract K=64 on PE row-groups
(0,0) and (64,0) and stream CONCURRENTLY into the two banks of one [128,1024]
PSUM tile, so a head-pair's scores cost one stream instead of two. One
1024-wide exp covers both heads. V tiles are padded to 128 columns (64 v + 32
ones for the softmax denominator + 32 zeros) so every stationary load takes
the fast-weight-load path. Output partials are fp16, summed on the host.
"""
import numpy as np

import concourse.bass as bass
import concourse.tile as tile
from concourse import bacc, mybir, bass_utils

F16 = mybir.dt.float16
F32 = mybir.dt.float32
EXP = mybir.ActivationFunctionType.Exp
IDENT = mybir.ActivationFunctionType.Identity

# Problem shape (hardcoded per harness contract)
B, LQ, D = 2, 2048, 1024
DCTX = 768
NH, HD = 16, 64
SCALE = 1.0 / 8.0  # 1/sqrt(64)

# Per-core shard: 4 heads (one group), one batch
GH = 4                # heads per core
ONES = 32             # d-replication rows per head
VW = 128              # per-head v_t width: 64 v + 32 ones + 32 zero pad (FWL)
VAW = GH * VW         # 512
GD = GH * HD          # 256: real v columns per chunk
KT_Q = D // 128       # 8
KT_C = DCTX // 128    # 6
NLK = LQ // 128       # 16 key tiles
NS = LQ // 512        # 4 query 512-slices
HALF = 1024
OPS_SLOT = {2: 0, 7: 1, 10: 2, 14: 3}


def _build():
    nc = bacc.Bacc("TRN2", target_bir_lowering=False, debug=False,
                   enable_asserts=False, num_devices=8)

    xT_d = nc.dram_tensor("xT", (D, LQ), F16, kind="ExternalInput").ap()
    cT_d = nc.dram_tensor("ctxT", (DCTX, LQ), F16, kind="ExternalInput").ap()
    wq_d = nc.dram_tensor("wq", (D, 256), F16, kind="ExternalInput").ap()
    wk_d = nc.dram_tensor("wk", (DCTX, 256), F16, kind="ExternalInput").ap()
    wv_d = nc.dram_tensor("wv", (DCTX, GD), F16, kind="ExternalInput").ap()
    wo_d = nc.dram_tensor("wo", (256, D), F16, kind="ExternalInput").ap()
    bq_d = nc.dram_tensor("bq", (128, 2), F32, kind="ExternalInput").ap()
    bk_d = nc.dram_tensor("bk", (128, 2), F32, kind="ExternalInput").ap()
    bvb_d = nc.dram_tensor("bvb", (128, GD), F32, kind="ExternalInput").ap()
    out_d = nc.dram_tensor("outT", (D, LQ), F16, kind="ExternalOutput").ap()

    with tile.TileContext(nc) as tc:
        with tc.tile_pool(name="w", bufs=1) as wp, \
             tc.tile_pool(name="xt", bufs=10) as xtp, \
             tc.tile_pool(name="ct", bufs=24) as ctp, \
             tc.tile_pool(name="act", bufs=1) as actp, \
             tc.tile_pool(name="expp", bufs=3) as expp, \
             tc.tile_pool(name="scrp", bufs=3) as scrp, \
             tc.tile_pool(name="rdp", bufs=3) as rdp, \
             tc.tile_pool(name="outp", bufs=3) as outp, \
             tc.tile_pool(name="ps_w", bufs=2, space="PSUM") as ps_w, \
             tc.tile_pool(name="ps_s", bufs=2, space="PSUM") as ps_s, \
             tc.tile_pool(name="ps_w2", bufs=1, space="PSUM") as ps_w2, \
             tc.tile_pool(name="ps_o", bufs=1, space="PSUM") as ps_o:

            # ---- weight/bias tiles (DMAs issued interleaved below) ----
            wq_t = wp.tile([128, KT_Q * 256], F16, tag="wq")
            wk_t = wp.tile([128, KT_C * 256], F16, tag="wk")
            wv_t = wp.tile([128, KT_C * GD], F16, tag="wv")
            wo_t = wp.tile([128, 2 * D], F16, tag="wo")
            bq_t = wp.tile([128, 2], F32, tag="bq")
            bk_t = wp.tile([128, 2], F32, tag="bk")
            bvb_t = wp.tile([128, GD], F32, tag="bvb")

            # K proj needs these first
            nc.sync.dma_start(wk_t[:].rearrange("p (kt m) -> p kt m", m=256),
                              wk_d.rearrange("(kt p) m -> p kt m", p=128))
            nc.sync.dma_start(bk_t[:], bk_d[:])

            # ---- persistent activation tiles ----
            qT = [actp.tile([128, LQ], F16, tag=f"qT{p}", name=f"qT{p}")
                  for p in range(2)]
            kT = [actp.tile([128, LQ], F16, tag=f"kT{p}", name=f"kT{p}")
                  for p in range(2)]
            v_t = actp.tile([128, NLK * VAW], F16, tag="v")
            aT = [actp.tile([128, LQ], F16, tag=f"aT{p}", name=f"aT{p}")
                  for p in range(2)]

            # constant ones (softmax denominator) / zero-pad rows of v_t
            v4 = v_t[:].rearrange("p (j w) -> p j w", w=VAW)
            for h in range(GH):
                nc.vector.memset(v4[:, :, VW * h + HD:VW * h + HD + ONES], 1.0)
                nc.vector.memset(v4[:, :, VW * h + HD + ONES:VW * (h + 1)], 0.0)

            # ---- K+V projections interleaved per ctx 512-slice ----
            ct_tiles = {}
            for s in range(NS):
                for kt in range(KT_C):
                    t = ctp.tile([128, 512], F16, tag="ct")
                    nc.sync.dma_start(
                        t[:], cT_d[128 * kt:128 * (kt + 1), 512 * s:512 * (s + 1)])
                    ct_tiles[(kt, s)] = t
                # stagger the remaining input loads behind the ct slices
                if s == 0:
                    nc.sync.dma_start(
                        wv_t[:].rearrange("p (kt m) -> p kt m", m=GD),
                        wv_d.rearrange("(kt p) m -> p kt m", p=128))
                    nc.sync.dma_start(bvb_t[:], bvb_d[:])
                elif s == 1:
                    nc.sync.dma_start(
                        wq_t[:].rearrange("p (kt m) -> p kt m", m=256),
                        wq_d.rearrange("(kt p) m -> p kt m", p=128))
                    nc.sync.dma_start(bq_t[:], bq_d[:])
                elif s == 2:
                    nc.sync.dma_start(
                        wo_t[:].rearrange("p (p2 m) -> p p2 m", m=1024),
                        wo_d.rearrange("(p2 p) m -> p p2 m", p=128))
                for p in range(2):
                    ps = ps_w.tile([128, 512], F32, tag="mm")
                    for kt in range(KT_C):
                        nc.tensor.matmul(
                            ps[:], wk_t[:, 256 * kt + 128 * p:256 * kt + 128 * (p + 1)],
                            ct_tiles[(kt, s)][:],
                            start=(kt == 0), stop=(kt == KT_C - 1))
                    nc.scalar.activation(
                        kT[p][:, 512 * s:512 * (s + 1)], ps[:], IDENT,
                        bias=bk_t[:, p:p + 1])
                for jj in range(4):
                    j = 4 * s + jj
                    ps = ps_w.tile([128, 512], F32, tag="mm")
                    for kt in range(KT_C):
                        nc.tensor.matmul(
                            ps[:, 0:GD],
                            ct_tiles[(kt, s)][:, 128 * jj:128 * (jj + 1)],
                            wv_t[:, GD * kt:GD * (kt + 1)],
                            start=(kt == 0), stop=(kt == KT_C - 1))
                    for h in range(GH):
                        nc.vector.tensor_add(
                            v_t[:, VAW * j + VW * h:VAW * j + VW * h + HD],
                            ps[:, HD * h:HD * (h + 1)],
                            bvb_t[:, HD * h:HD * (h + 1)])

            # ---- Q projection; slice 0 up front, rest interleaved into
            # attention (q-slice si+1 computed during attention on si)
            def q_proj(s, pp):
                xt_tiles = []
                for kt in range(KT_Q):
                    t = xtp.tile([128, 512], F16, tag="xt",
                                 name=f"xt{s}_{pp}_{kt}")
                    nc.sync.dma_start(
                        t[:], xT_d[128 * kt:128 * (kt + 1), 512 * s:512 * (s + 1)])
                    xt_tiles.append(t)
                ps = ps_o.tile([128, 512], F32, tag="o", name=f"qps{s}_{pp}")
                for kt in range(KT_Q):
                    nc.tensor.matmul(
                        ps[:], wq_t[:, 256 * kt + 128 * pp:256 * kt + 128 * (pp + 1)],
                        xt_tiles[kt][:],
                        start=(kt == 0), stop=(kt == KT_Q - 1))
                qsc = scrp.tile([128, 512], F32, tag="scr", name=f"qsc{s}_{pp}")
                nc.vector.tensor_scalar_add(qsc[:], ps[:], bq_t[:, pp:pp + 1])
                nc.vector.tensor_copy(qT[pp][:, 512 * s:512 * (s + 1)], qsc[:])

            for pp in range(2):
                q_proj(0, pp)

            def out_proj_group(mo, s, pool=None, on_act=False):
                pool = pool if pool is not None else ps_o
                tg = "o" if pool is ps_o else ("mm" if pool is ps_w else "mm2")
                ops = pool.tile([128, 512], F32, tag=tg, name=f"ops{mo}_{s}")
                for p in range(2):
                    nc.tensor.matmul(
                        ops[:], wo_t[:, D * p + 128 * mo:D * p + 128 * (mo + 1)],
                        aT[p][:, 512 * s:512 * (s + 1)],
                        start=(p == 0), stop=(p == 1))
                ot = outp.tile([128, 512], F16, tag="out")
                if on_act:
                    nc.scalar.copy(ot[:], ops[:])
                else:
                    nc.vector.tensor_copy(ot[:], ops[:])
                nc.sync.dma_start(
                    out_d[128 * mo:128 * (mo + 1), 512 * s:512 * (s + 1)], ot[:])

            # ---- attention: q-slice outer; slice s-1's output projection
            # interleaved into slice s's j-loop to keep the exp stream dense
            for si in range(NS):
                cols = slice(512 * si, 512 * (si + 1))
                for p in range(2):
                    hA, hB = 2 * p, 2 * p + 1
                    pa = {0: ps_w.tile([128, 512], F32, tag="mm",
                                       name=f"pa{si}_{p}_0"),
                          1: ps_w2.tile([128, 512], F32, tag="mm2",
                                        name=f"pa{si}_{p}_1")}
                    for j in range(NLK):
                        ks = slice(128 * j, 128 * (j + 1))
                        st = ps_s.tile([128, HALF], F32, tag="s")
                        # concurrent PE row-group pair: head A rows 0:64,
                        # head B rows 64:128, disjoint PSUM banks
                        nc.tensor.matmul(
                            st[:, 0:512], kT[p][0:64, ks], qT[p][0:64, cols],
                            start=True, stop=True)
                        nc.tensor.matmul(
                            st[:, 512:1024], kT[p][64:128, ks],
                            qT[p][64:128, cols], start=True, stop=True)
                        ex = expp.tile([128, HALF], F16, tag="expS")
                        nc.scalar.activation(ex[:], st[:], EXP, scale=SCALE)
                        nc.tensor.matmul(
                            pa[0][:],
                            v_t[:, VAW * j + VW * hA:VAW * j + VW * (hA + 1)],
                            ex[:, 0:512],
                            start=(j == 0), stop=(j == NLK - 1))
                        nc.tensor.matmul(
                            pa[1][:],
                            v_t[:, VAW * j + VW * hB:VAW * j + VW * (hB + 1)],
                            ex[:, 512:1024],
                            start=(j == 0), stop=(j == NLK - 1))
                        if si > 0 and j in (2, 7, 10, 14):
                            out_proj_group(4 * p + OPS_SLOT[j], si - 1)
                        if si < NS - 1 and j == 4:
                            q_proj(si + 1, p)
                    # normalize: attnT = attnU * (1/d); d-block replicated to
                    # 64 partitions via PSUM->SBUF shifts (SBUF->SBUF illegal)
                    for hh in (1, 0):
                        r0 = 64 * hh
                        pan = pa[hh]
                        dsb = rdp.tile([64, 512], F32, tag="dsb")
                        nc.vector.tensor_copy(dsb[0:32, :], pan[64:96, :])
                        nc.vector.tensor_copy(dsb[32:64, :], pan[64:96, :])
                        rd = rdp.tile([64, 512], F32, tag="rd")
                        rds = rdp.tile([64, 512], F32, tag="rds")
                        if hh == 1:
                            # single-buffered pool: copy out fast to free it
                            scr = scrp.tile([64, 512], F32, tag="scr",
                                            name=f"scr{si}_{p}")
                            nc.vector.tensor_copy(scr[:], pan[0:64, :])
                            src_rows = scr[:]
                        else:
                            src_rows = pan[0:64, :]
                        nc.vector.reciprocal_approx_accurate(
                            rd[:], dsb[:], rds[:])
                        nc.vector.tensor_mul(
                            aT[p][r0:r0 + 64, cols], src_rows, rd[:])
            for mo in range(D // 128):
                out_proj_group(mo, NS - 1,
                               pool=(ps_o if mo % 2 == 0 else ps_w2),
                               on_act=(mo % 2 == 1))

    nc.compile()
    return nc


_NC_CACHE = []


def _get_nc():
    if not _NC_CACHE:
        _NC_CACHE.append(_build())
    return _NC_CACHE[0]


def kernel_run(inputs, trace=False, **kw):
    """Run on HW; returns (full_output, BassKernelResults)."""
    x = np.asarray(inputs["x"], np.float32)
    context = np.asarray(inputs["context"], np.float32)
    w_q = np.asarray(inputs["w_q"], np.float32)
    b_q = np.asarray(inputs["b_q"], np.float32)
    w_k = np.asarray(inputs["w_k"], np.float32)
    b_k = np.asarray(inputs["b_k"], np.float32)
    w_v = np.asarray(inputs["w_v"], np.float32)
    b_v = np.asarray(inputs["b_v"], np.float32)
    w_o = np.asarray(inputs["w_o"], np.float32)
    b_o = np.asarray(inputs["b_o"], np.float32)

    f16 = np.float16
    xT_h = [np.ascontiguousarray(x[b].T).astype(f16) for b in range(B)]
    cT_h = [np.ascontiguousarray(context[b].T).astype(f16) for b in range(B)]

    maps = []
    for c in range(8):
        b, g = c // 4, c % 4
        hs = slice(256 * g, 256 * (g + 1))
        maps.append({
            "xT": xT_h[b],
            "ctxT": cT_h[b],
            "wq": np.ascontiguousarray(w_q[:, hs]).astype(f16),
            "wk": np.ascontiguousarray(w_k[:, hs]).astype(f16),
            "wv": np.ascontiguousarray(w_v[:, hs]).astype(f16),
            "wo": np.ascontiguousarray(w_o[hs, :]).astype(f16),
            "bq": np.ascontiguousarray(b_q[hs].reshape(2, 128).T),
            "bk": np.ascontiguousarray(b_k[hs].reshape(2, 128).T),
            "bvb": np.ascontiguousarray(
                np.broadcast_to(b_v[None, hs], (128, GD)).astype(np.float32)),
        })

    nc = _get_nc()
    res = bass_utils.run_bass_kernel_spmd(nc, maps, core_ids=list(range(8)),
                                          trace=trace, **kw)
    out = np.empty((B, LQ, D), np.float32)
    for b in range(B):
        acc = res.results[4 * b]["outT"].astype(np.float32)
        for g in range(1, 4):
            acc = acc + res.results[4 * b + g]["outT"].astype(np.float32)
        out[b] = acc.T + b_o[None, :]
    return out, res


def kernel(**inputs) -> np.ndarray:
    out, _ = kernel_run(inputs)
    return out


# revision 20
# speedup vs baseline: 1.1136x; 1.0230x over previous
"""Cross-attention Trainium2 kernel (nn_CrossAttention, B=2, L=2048, D=1024,
Dctx=768, 16 heads x 64).

Sharding: 8 cores = 2 (batch) x 4 (head-groups of 4 heads). Each core computes
its batch's Q/K/V projections for its 4 heads, flash-style attention in the
transposed (S^T) domain, and a partial output projection; the host sums the
head-group partials and adds b_o.

All activations live transposed on-chip (xT, ctxT, qT, kT, attnT) so every
matmul contracts over the partition dim with no on-chip transposes; operands
are fp16 (full PE streaming rate) with fp32 PSUM accumulation. Heads are
processed in pairs: the pair's scores matmuls contract K=64 on PE row-groups
(0,0) and (64,0) and stream CONCURRENTLY into the two banks of one [128,1024]
PSUM tile, so a head-pair's scores cost one stream instead of two. One
1024-wide exp covers both heads. V tiles are padded to 128 columns (64 v + 32
ones for the softmax denominator + 32 zeros) so every stationary load takes
the fast-weight-load path. Output partials are fp16, summed on the host.
"""
import numpy as np

import concourse.bass as bass
import concourse.tile as tile
from concourse import bacc, mybir, bass_utils

F16 = mybir.dt.float16
F32 = mybir.dt.float32
EXP = mybir.ActivationFunctionType.Exp
IDENT = mybir.ActivationFunctionType.Identity

# Problem shape (hardcoded per harness contract)
B, LQ, D = 2, 2048, 1024
DCTX = 768
NH, HD = 16, 64
SCALE = 1.0 / 8.0  # 1/sqrt(64)

# Per-core shard: 4 heads (one group), one batch
GH = 4                # heads per core
ONES = 32             # d-replication rows per head
VW = 128              # per-head v_t width: 64 v + 32 ones + 32 zero pad (FWL)
VAW = GH * VW         # 512
GD = GH * HD          # 256: real v columns per chunk
KT_Q = D // 128       # 8
KT_C = DCTX // 128    # 6
NLK = LQ // 128       # 16 key tiles
NS = LQ // 512        # 4 query 512-slices
HALF = 1024
OPS_SLOT = {2: 0, 7: 1, 10: 2, 14: 3}


def _build():
    nc = bacc.Bacc("TRN2", target_bir_lowering=False, debug=False,
                   enable_asserts=False, num_devices=8)

    xT_d = nc.dram_tensor("xT", (D, LQ), F16, kind="ExternalInput").ap()
    cT_d = nc.dram_tensor("ctxT", (DCTX, LQ), F16, kind="ExternalInput").ap()
    wq_d = nc.dram_tensor("wq", (D, 256), F16, kind="ExternalInput").ap()
    wk_d = nc.dram_tensor("wk", (DCTX, 256), F16, kind="ExternalInput").ap()
    wv_d = nc.dram_tensor("wv", (DCTX, GD), F16, kind="ExternalInput").ap()
    wo_d = nc.dram_tensor("wo", (256, D), F16, kind="ExternalInput").ap()
    bq_d = nc.dram_tensor("bq", (128, 2), F32, kind="ExternalInput").ap()
    bk_d = nc.dram_tensor("bk", (128, 2), F32, kind="ExternalInput").ap()
    bvb_d = nc.dram_tensor("bvb", (128, GD), F32, kind="ExternalInput").ap()
    out_d = nc.dram_tensor("outT", (D, LQ), F16, kind="ExternalOutput").ap()

    with tile.TileContext(nc) as tc:
        with tc.tile_pool(name="w", bufs=1) as wp, \
             tc.tile_pool(name="xt", bufs=10) as xtp, \
             tc.tile_pool(name="ct", bufs=24) as ctp, \
             tc.tile_pool(name="act", bufs=1) as actp, \
             tc.tile_pool(name="expp", bufs=3) as expp, \
             tc.tile_pool(name="scrp", bufs=3) as scrp, \
             tc.tile_pool(name="rdp", bufs=3) as rdp, \
             tc.tile_pool(name="outp", bufs=3) as outp, \
             tc.tile_pool(name="ps_w", bufs=2, space="PSUM") as ps_w, \
             tc.tile_pool(name="ps_s", bufs=2, space="PSUM") as ps_s, \
             tc.tile_pool(name="ps_w2", bufs=1, space="PSUM") as ps_w2, \
             tc.tile_pool(name="ps_o", bufs=1, space="PSUM") as ps_o:

            # ---- weight/bias tiles (DMAs issued interleaved below) ----
            wq_t = wp.tile([128, KT_Q * 256], F16, tag="wq")
            wk_t = wp.tile([128, KT_C * 256], F16, tag="wk")
            wv_t = wp.tile([128, KT_C * GD], F16, tag="wv")
            wo_t = wp.tile([128, 2 * D], F16, tag="wo")
            bq_t = wp.tile([128, 2], F32, tag="bq")
            bk_t = wp.tile([128, 2], F32, tag="bk")
            bvb_t = wp.tile([128, GD], F32, tag="bvb")

            # K proj needs these first
            nc.sync.dma_start(wk_t[:].rearrange("p (kt m) -> p kt m", m=256),
                              wk_d.rearrange("(kt p) m -> p kt m", p=128))
            nc.sync.dma_start(bk_t[:], bk_d[:])

            # ---- persistent activation tiles ----
            qT = [actp.tile([128, LQ], F16, tag=f"qT{p}", name=f"qT{p}")
                  for p in range(2)]
            kT = [actp.tile([128, LQ], F16, tag=f"kT{p}", name=f"kT{p}")
                  for p in range(2)]
            v_t = actp.tile([128, NLK * VAW], F16, tag="v")
            aT = [actp.tile([128, LQ], F16, tag=f"aT{p}", name=f"aT{p}")
                  for p in range(2)]

            # constant ones (softmax denominator) / zero-pad rows of v_t
            v4 = v_t[:].rearrange("p (j w) -> p j w", w=VAW)
            for h in range(GH):
                nc.vector.memset(v4[:, :, VW * h + HD:VW * h + HD + ONES], 1.0)
                nc.vector.memset(v4[:, :, VW * h + HD + ONES:VW * (h + 1)], 0.0)

            # ---- K+V projections interleaved per ctx 512-slice ----
            ct_tiles = {}
            for s in range(NS):
                for kt in range(KT_C):
                    t = ctp.tile([128, 512], F16, tag="ct")
                    nc.sync.dma_start(
                        t[:], cT_d[128 * kt:128 * (kt + 1), 512 * s:512 * (s + 1)])
                    ct_tiles[(kt, s)] = t
                # stagger the remaining input loads behind the ct slices
                if s == 0:
                    nc.sync.dma_start(
                        wv_t[:].rearrange("p (kt m) -> p kt m", m=GD),
                        wv_d.rearrange("(kt p) m -> p kt m", p=128))
                    nc.sync.dma_start(bvb_t[:], bvb_d[:])
                elif s == 1:
                    nc.sync.dma_start(
                        wq_t[:].rearrange("p (kt m) -> p kt m", m=256),
                        wq_d.rearrange("(kt p) m -> p kt m", p=128))
                    nc.sync.dma_start(bq_t[:], bq_d[:])
                elif s == 2:
                    nc.sync.dma_start(
                        wo_t[:].rearrange("p (p2 m) -> p p2 m", m=1024),
                        wo_d.rearrange("(p2 p) m -> p p2 m", p=128))
                for p in range(2):
                    ps = ps_w.tile([128, 512], F32, tag="mm")
                    for kt in range(KT_C):
                        nc.tensor.matmul(
                            ps[:], wk_t[:, 256 * kt + 128 * p:256 * kt + 128 * (p + 1)],
                            ct_tiles[(kt, s)][:],
                            start=(kt == 0), stop=(kt == KT_C - 1))
                    nc.scalar.activation(
                        kT[p][:, 512 * s:512 * (s + 1)], ps[:], IDENT,
                        bias=bk_t[:, p:p + 1])
                for jj in range(4):
                    j = 4 * s + jj
                    ps = ps_w.tile([128, 512], F32, tag="mm")
                    for kt in range(KT_C):
                        nc.tensor.matmul(
                            ps[:, 0:GD],
                            ct_tiles[(kt, s)][:, 128 * jj:128 * (jj + 1)],
                            wv_t[:, GD * kt:GD * (kt + 1)],
                            start=(kt == 0), stop=(kt == KT_C - 1))
                    for h in range(GH):
                        nc.vector.tensor_add(
                            v_t[:, VAW * j + VW * h:VAW * j + VW * h + HD],
                            ps[:, HD * h:HD * (h + 1)],
                            bvb_t[:, HD * h:HD * (h + 1)])

            # ---- Q projection; slice 0 up front, rest interleaved into
            # attention (q-slice si+1 computed during attention on si)
            def q_proj(s, pp):
                xt_tiles = []
                for kt in range(KT_Q):
                    t = xtp.tile([128, 512], F16, tag="xt",
                                 name=f"xt{s}_{pp}_{kt}")
                    nc.sync.dma_start(
                        t[:], xT_d[128 * kt:128 * (kt + 1), 512 * s:512 * (s + 1)])
                    xt_tiles.append(t)
                ps = ps_o.tile([128, 512], F32, tag="o", name=f"qps{s}_{pp}")
                for kt in range(KT_Q):
                    nc.tensor.matmul(
                        ps[:], wq_t[:, 256 * kt + 128 * pp:256 * kt + 128 * (pp + 1)],
                        xt_tiles[kt][:],
                        start=(kt == 0), stop=(kt == KT_Q - 1))
                qsc = scrp.tile([128, 512], F32, tag="scr", name=f"qsc{s}_{pp}")
                nc.vector.tensor_scalar_add(qsc[:], ps[:], bq_t[:, pp:pp + 1])
                nc.vector.tensor_copy(qT[pp][:, 512 * s:512 * (s + 1)], qsc[:])

            for pp in range(2):
                q_proj(0, pp)

            def out_proj_group(mo, s, pool=None, on_act=False):
                pool = pool if pool is not None else ps_o
                tg = "o" if pool is ps_o else ("mm" if pool is ps_w else "mm2")
                ops = pool.tile([128, 512], F32, tag=tg, name=f"ops{mo}_{s}")
                for p in range(2):
                    nc.tensor.matmul(
                        ops[:], wo_t[:, D * p + 128 * mo:D * p + 128 * (mo + 1)],
                        aT[p][:, 512 * s:512 * (s + 1)],
                        start=(p == 0), stop=(p == 1))
                ot = outp.tile([128, 512], F16, tag="out")
                if on_act:
                    nc.scalar.copy(ot[:], ops[:])
                else:
                    nc.vector.tensor_copy(ot[:], ops[:])
                nc.sync.dma_start(
                    out_d[128 * mo:128 * (mo + 1), 512 * s:512 * (s + 1)], ot[:])

            # ---- attention: q-slice outer; slice s-1's output projection
            # interleaved into slice s's j-loop to keep the exp stream dense
            for si in range(NS):
                cols = slice(512 * si, 512 * (si + 1))
                for p in range(2):
                    hA, hB = 2 * p, 2 * p + 1
                    pa = {0: ps_w.tile([128, 512], F32, tag="mm",
                                       name=f"pa{si}_{p}_0"),
                          1: ps_w2.tile([128, 512], F32, tag="mm2",
                                        name=f"pa{si}_{p}_1")}
                    for j in range(NLK):
                        ks = slice(128 * j, 128 * (j + 1))
                        st = ps_s.tile([128, HALF], F32, tag="s")
                        # concurrent PE row-group pair: head A rows 0:64,
                        # head B rows 64:128, disjoint PSUM banks
                        nc.tensor.matmul(
                            st[:, 0:512], kT[p][0:64, ks], qT[p][0:64, cols],
                            start=True, stop=True)
                        nc.tensor.matmul(
                            st[:, 512:1024], kT[p][64:128, ks],
                            qT[p][64:128, cols], start=True, stop=True)
                        ex = expp.tile([128, HALF], F16, tag="expS")
                        nc.scalar.activation(ex[:], st[:], EXP, scale=SCALE)
                        nc.tensor.matmul(
                            pa[0][:],
                            v_t[:, VAW * j + VW * hA:VAW * j + VW * (hA + 1)],
                            ex[:, 0:512],
                            start=(j == 0), stop=(j == NLK - 1))
                        nc.tensor.matmul(
                            pa[1][:],
                            v_t[:, VAW * j + VW * hB:VAW * j + VW * (hB + 1)],
                            ex[:, 512:1024],
                            start=(j == 0), stop=(j == NLK - 1))
                        if si > 0 and j in (2, 7, 10, 14):
                            out_proj_group(4 * p + OPS_SLOT[j], si - 1)
                        if si < NS - 1 and j == 4:
                            q_proj(si + 1, p)
                    # normalize: attnT = attnU * (1/d); d-block replicated to
                    # 64 partitions via PSUM->SBUF shifts (SBUF->SBUF illegal)
                    for hh in (1, 0):
                        r0 = 64 * hh
                        pan = pa[hh]
                        dsb = rdp.tile([64, 512], F32, tag="dsb")
                        nc.vector.tensor_copy(dsb[0:32, :], pan[64:96, :])
                        nc.vector.tensor_copy(dsb[32:64, :], pan[64:96, :])
                        rd = rdp.tile([64, 512], F32, tag="rd")
                        rds = rdp.tile([64, 512], F32, tag="rds")
                        if hh == 1:
                            # single-buffered pool: copy out fast to free it
                            scr = scrp.tile([64, 512], F32, tag="scr",
                                            name=f"scr{si}_{p}")
                            nc.vector.tensor_copy(scr[:], pan[0:64, :])
                            src_rows = scr[:]
                        else:
                            src_rows = pan[0:64, :]
                        nc.vector.reciprocal_approx_accurate(
                            rd[:], dsb[:], rds[:])
                        if hh == 1:
                            nc.gpsimd.tensor_mul(
                                aT[p][r0:r0 + 64, cols], src_rows, rd[:])
                        else:
                            nc.vector.tensor_mul(
                                aT[p][r0:r0 + 64, cols], src_rows, rd[:])
            for mo in range(D // 128):
                out_proj_group(mo, NS - 1,
                               pool=(ps_o if mo % 2 == 0 else ps_w2),
                               on_act=(mo % 2 == 1))

    nc.compile()
    return nc


_NC_CACHE = []


def _get_nc():
    if not _NC_CACHE:
        _NC_CACHE.append(_build())
    return _NC_CACHE[0]


def kernel_run(inputs, trace=False, **kw):
    """Run on HW; returns (full_output, BassKernelResults)."""
    x = np.asarray(inputs["x"], np.float32)
    context = np.asarray(inputs["context"], np.float32)
    w_q = np.asarray(inputs["w_q"], np.float32)
    b_q = np.asarray(inputs["b_q"], np.float32)
    w_k = np.asarray(inputs["w_k"], np.float32)
    b_k = np.asarray(inputs["b_k"], np.float32)
    w_v = np.asarray(inputs["w_v"], np.float32)
    b_v = np.asarray(inputs["b_v"], np.float32)
    w_o = np.asarray(inputs["w_o"], np.float32)
    b_o = np.asarray(inputs["b_o"], np.float32)

    f16 = np.float16
    xT_h = [np.ascontiguousarray(x[b].T).astype(f16) for b in range(B)]
    cT_h = [np.ascontiguousarray(context[b].T).astype(f16) for b in range(B)]

    maps = []
    for c in range(8):
        b, g = c // 4, c % 4
        hs = slice(256 * g, 256 * (g + 1))
        maps.append({
            "xT": xT_h[b],
            "ctxT": cT_h[b],
            "wq": np.ascontiguousarray(w_q[:, hs]).astype(f16),
            "wk": np.ascontiguousarray(w_k[:, hs]).astype(f16),
            "wv": np.ascontiguousarray(w_v[:, hs]).astype(f16),
            "wo": np.ascontiguousarray(w_o[hs, :]).astype(f16),
            "bq": np.ascontiguousarray(b_q[hs].reshape(2, 128).T),
            "bk": np.ascontiguousarray(b_k[hs].reshape(2, 128).T),
            "bvb": np.ascontiguousarray(
                np.broadcast_to(b_v[None, hs], (128, GD)).astype(np.float32)),
        })

    nc = _get_nc()
    res = bass_utils.run_bass_kernel_spmd(nc, maps, core_ids=list(range(8)),
                                          trace=trace, **kw)
    out = np.empty((B, LQ, D), np.float32)
    for b in range(B):
        acc = res.results[4 * b]["outT"].astype(np.float32)
        for g in range(1, 4):
            acc = acc + res.results[4 * b + g]["outT"].astype(np.float32)
        out[b] = acc.T + b_o[None, :]
    return out, res


def kernel(**inputs) -> np.ndarray:
    out, _ = kernel_run(inputs)
    return out


# revision 38
# speedup vs baseline: 1.1850x; 1.0641x over previous
"""Cross-attention Trainium2 kernel (nn_CrossAttention, B=2, L=2048, D=1024,
Dctx=768, 16 heads x 64).

Sharding: 8 cores = 2 (batch) x 4 (head-groups of 4 heads). Each core computes
its batch's Q/K/V projections for its 4 heads, flash-style attention in the
transposed (S^T) domain, and a partial output projection; the host sums the
head-group partials and adds b_o.

All activations live transposed on-chip (xT, ctxT, qT, kT, attnT) so every
matmul contracts over the partition dim with no on-chip transposes; operands
are fp16 (full PE streaming rate) with fp32 PSUM accumulation. Heads are
processed in pairs: the pair's scores matmuls contract K=64 on PE row-groups
(0,0) and (64,0) and stream CONCURRENTLY into the two banks of one [128,1024]
PSUM tile, so a head-pair's scores cost one stream instead of two. One
1024-wide exp covers both heads. V tiles are padded to 128 columns (64 v + 32
ones for the softmax denominator + 32 zeros) so every stationary load takes
the fast-weight-load path. Output partials are fp16, summed on the host.
"""
import numpy as np

import concourse.bass as bass
import concourse.tile as tile
from concourse import bacc, mybir, bass_utils

F16 = mybir.dt.float16
F32 = mybir.dt.float32
EXP = mybir.ActivationFunctionType.Exp
IDENT = mybir.ActivationFunctionType.Identity

# Problem shape (hardcoded per harness contract)
B, LQ, D = 2, 2048, 1024
DCTX = 768
NH, HD = 16, 64
SCALE = 1.0 / 8.0  # 1/sqrt(64)

# Per-core shard: 4 heads (one group), one batch
GH = 4                # heads per core
ONES = 64             # d-replication rows per head
VW = 128              # per-head v_t width: 64 v + 64 ones (FWL needs 128 cols)
VAW = GH * VW         # 512
GD = GH * HD          # 256: real v columns per chunk
KT_Q = D // 128       # 8
KT_C = DCTX // 128    # 6
NLK = LQ // 128       # 16 key tiles
NS = LQ // 512        # 4 query 512-slices
HALF = 1024
OPS_SLOT = {2: 0, 7: 1, 10: 2, 14: 3}


def _build():
    nc = bacc.Bacc("TRN2", target_bir_lowering=False, debug=False,
                   enable_asserts=False, num_devices=8)

    xT_d = nc.dram_tensor("xT", (D, LQ), F16, kind="ExternalInput").ap()
    cT_d = nc.dram_tensor("ctxT", (DCTX, LQ), F16, kind="ExternalInput").ap()
    wq_d = nc.dram_tensor("wq", (D, 256), F16, kind="ExternalInput").ap()
    wk_d = nc.dram_tensor("wk", (DCTX, 256), F16, kind="ExternalInput").ap()
    wv_d = nc.dram_tensor("wv", (DCTX, GD), F16, kind="ExternalInput").ap()
    wo_d = nc.dram_tensor("wo", (256, D), F16, kind="ExternalInput").ap()
    bq_d = nc.dram_tensor("bq", (128, 2), F32, kind="ExternalInput").ap()
    bk_d = nc.dram_tensor("bk", (128, 2), F32, kind="ExternalInput").ap()
    bvb_d = nc.dram_tensor("bvb", (128, GD), F32, kind="ExternalInput").ap()
    out_d = nc.dram_tensor("outT", (D, LQ), F16, kind="ExternalOutput").ap()

    with tile.TileContext(nc) as tc:
        with tc.tile_pool(name="w", bufs=1) as wp, \
             tc.tile_pool(name="xt", bufs=18) as xtp, \
             tc.tile_pool(name="ct", bufs=24) as ctp, \
             tc.tile_pool(name="act", bufs=1) as actp, \
             tc.tile_pool(name="expp", bufs=6) as expp, \
             tc.tile_pool(name="scrp", bufs=3) as scrp, \
             tc.tile_pool(name="rdp", bufs=4) as rdp, \
             tc.tile_pool(name="outp", bufs=3) as outp, \
             tc.tile_pool(name="ps_w", bufs=2, space="PSUM") as ps_w, \
             tc.tile_pool(name="ps_s", bufs=2, space="PSUM") as ps_s, \
             tc.tile_pool(name="ps_w2", bufs=1, space="PSUM") as ps_w2, \
             tc.tile_pool(name="ps_o", bufs=1, space="PSUM") as ps_o:

            # ---- weight/bias tiles (DMAs issued interleaved below) ----
            wq_t = wp.tile([128, KT_Q * 256], F16, tag="wq")
            wk_t = wp.tile([128, KT_C * 256], F16, tag="wk")
            wv_t = wp.tile([128, KT_C * GD], F16, tag="wv")
            wo_t = wp.tile([128, 2 * D], F16, tag="wo")
            bq_t = wp.tile([128, 2], F32, tag="bq")
            bk_t = wp.tile([128, 2], F32, tag="bk")
            bvb_t = wp.tile([128, GD], F32, tag="bvb")

            # K proj needs these first
            nc.sync.dma_start(wk_t[:].rearrange("p (kt m) -> p kt m", m=256),
                              wk_d.rearrange("(kt p) m -> p kt m", p=128))
            nc.sync.dma_start(bk_t[:], bk_d[:])

            # ---- persistent activation tiles ----
            qT = [actp.tile([128, LQ], F16, tag=f"qT{p}", name=f"qT{p}")
                  for p in range(2)]
            kT = [actp.tile([128, LQ], F16, tag=f"kT{p}", name=f"kT{p}")
                  for p in range(2)]
            v_t = actp.tile([128, NLK * VAW], F16, tag="v")
            aT = [actp.tile([128, LQ], F16, tag=f"aT{p}", name=f"aT{p}")
                  for p in range(2)]

            # constant ones (softmax denominator) / zero-pad rows of v_t
            v4 = v_t[:].rearrange("p (j w) -> p j w", w=VAW)
            for h in range(GH):
                nc.vector.memset(v4[:, :, VW * h + HD:VW * (h + 1)], 1.0)

            # ---- K+V projections interleaved per ctx 512-slice ----
            head_ps_rot = [0]
            ct_tiles = {}
            for s in range(NS):
                for kt in range(KT_C):
                    t = ctp.tile([128, 512], F16, tag="ct")
                    nc.sync.dma_start(
                        t[:], cT_d[128 * kt:128 * (kt + 1), 512 * s:512 * (s + 1)])
                    ct_tiles[(kt, s)] = t
                # stagger the remaining input loads behind the ct slices
                if s == 0:
                    nc.sync.dma_start(
                        wv_t[:].rearrange("p (kt m) -> p kt m", m=GD),
                        wv_d.rearrange("(kt p) m -> p kt m", p=128))
                    nc.sync.dma_start(bvb_t[:], bvb_d[:])
                elif s == 1:
                    nc.sync.dma_start(
                        wq_t[:].rearrange("p (kt m) -> p kt m", m=256),
                        wq_d.rearrange("(kt p) m -> p kt m", p=128))
                    nc.sync.dma_start(bq_t[:], bq_d[:])
                elif s == 2:
                    nc.sync.dma_start(
                        wo_t[:].rearrange("p (p2 m) -> p p2 m", m=1024),
                        wo_d.rearrange("(p2 p) m -> p p2 m", p=128))
                    xt_dma(0)
                def head_ps(name):
                    k = head_ps_rot[0]
                    head_ps_rot[0] = (k + 1) % 2
                    if k == 1:
                        return ps_s.tile([128, HALF], F32, tag="s",
                                         name=name)[:, 0:512]
                    return ps_w.tile([128, 512], F32, tag="mm", name=name)[:]

                for p in range(2):
                    ps = head_ps(f"kps{s}_{p}")
                    for kt in range(KT_C):
                        nc.tensor.matmul(
                            ps, wk_t[:, 256 * kt + 128 * p:256 * kt + 128 * (p + 1)],
                            ct_tiles[(kt, s)][:],
                            start=(kt == 0), stop=(kt == KT_C - 1))
                    nc.scalar.activation(
                        kT[p][:, 512 * s:512 * (s + 1)], ps, IDENT,
                        bias=bk_t[:, p:p + 1])
                for jj in range(4):
                    j = 4 * s + jj
                    ps = head_ps(f"vps{j}")
                    for kt in range(KT_C):
                        nc.tensor.matmul(
                            ps[:, 0:GD],
                            ct_tiles[(kt, s)][:, 128 * jj:128 * (jj + 1)],
                            wv_t[:, GD * kt:GD * (kt + 1)],
                            start=(kt == 0), stop=(kt == KT_C - 1))
                    for h in range(GH):
                        nc.vector.tensor_add(
                            v_t[:, VAW * j + VW * h:VAW * j + VW * h + HD],
                            ps[:, HD * h:HD * (h + 1)],
                            bvb_t[:, HD * h:HD * (h + 1)])

            # ---- Q projection; slice 0 up front, rest interleaved into
            # attention (q-slice si+1 computed during attention on si)
            xt_cache = {}

            def xt_dma(s):
                tiles = []
                for kt in range(KT_Q):
                    t = xtp.tile([128, 512], F16, tag="xt",
                                 name=f"xt{s}_{kt}")
                    nc.sync.dma_start(
                        t[:], xT_d[128 * kt:128 * (kt + 1), 512 * s:512 * (s + 1)])
                    tiles.append(t)
                xt_cache[s] = tiles
                return tiles

            qps_live = {}

            def q_proj_chunk(s, pp, ci):
                xt_tiles = xt_cache.get(s) or xt_dma(s)
                if ci == 0:
                    qps_live[(s, pp)] = ps_o.tile(
                        [128, 512], F32, tag="o", name=f"qps{s}_{pp}")
                ps = qps_live[(s, pp)]
                for kt in range(2 * ci, 2 * ci + 2):
                    nc.tensor.matmul(
                        ps[:], wq_t[:, 256 * kt + 128 * pp:256 * kt + 128 * (pp + 1)],
                        xt_tiles[kt][:],
                        start=(kt == 0), stop=(kt == KT_Q - 1))
                if ci == 3:
                    qsc = scrp.tile([128, 512], F32, tag="scr",
                                    name=f"qsc{s}_{pp}")
                    nc.vector.tensor_scalar_add(qsc[:], ps[:], bq_t[:, pp:pp + 1])
                    nc.vector.tensor_copy(qT[pp][:, 512 * s:512 * (s + 1)], qsc[:])

            def q_proj(s, pp, on_act=False):
                xt_tiles = xt_cache.get(s) or xt_dma(s)
                ps = ps_o.tile([128, 512], F32, tag="o", name=f"qps{s}_{pp}")
                for kt in range(KT_Q):
                    nc.tensor.matmul(
                        ps[:], wq_t[:, 256 * kt + 128 * pp:256 * kt + 128 * (pp + 1)],
                        xt_tiles[kt][:],
                        start=(kt == 0), stop=(kt == KT_Q - 1))
                if on_act:
                    nc.scalar.activation(
                        qT[pp][:, 512 * s:512 * (s + 1)], ps[:], IDENT,
                        bias=bq_t[:, pp:pp + 1])
                else:
                    qsc = scrp.tile([128, 512], F32, tag="scr",
                                    name=f"qsc{s}_{pp}")
                    nc.vector.tensor_scalar_add(qsc[:], ps[:], bq_t[:, pp:pp + 1])
                    nc.vector.tensor_copy(qT[pp][:, 512 * s:512 * (s + 1)], qsc[:])

            for pp in range(2):
                q_proj(0, pp, on_act=True)

            def out_proj_group(mo, s, pool=None, on_act=False):
                pool = pool if pool is not None else ps_o
                if pool is ps_s:
                    ops = ps_s.tile([128, HALF], F32, tag="s",
                                    name=f"ops{mo}_{s}")[:, 0:512]
                else:
                    tg = "o" if pool is ps_o else ("mm" if pool is ps_w else "mm2")
                    ops = pool.tile([128, 512], F32, tag=tg, name=f"ops{mo}_{s}")
                for p in range(2):
                    nc.tensor.matmul(
                        ops, wo_t[:, D * p + 128 * mo:D * p + 128 * (mo + 1)],
                        aT[p][:, 512 * s:512 * (s + 1)],
                        start=(p == 0), stop=(p == 1))
                ot = outp.tile([128, 512], F16, tag="out")
                if on_act:
                    nc.scalar.copy(ot[:], ops)
                else:
                    nc.vector.tensor_copy(ot[:], ops)
                nc.sync.dma_start(
                    out_d[128 * mo:128 * (mo + 1), 512 * s:512 * (s + 1)], ot[:])

            pend = []

            def flush_attnv():
                while pend:
                    fpa, fa, fb, fj, fex = pend.pop()
                    nc.tensor.matmul(
                        fpa[0][:],
                        v_t[:, VAW * fj + VW * fa:VAW * fj + VW * (fa + 1)],
                        fex[:, 0:512],
                        start=(fj == 0), stop=(fj == NLK - 1))
                    nc.tensor.matmul(
                        fpa[1][:],
                        v_t[:, VAW * fj + VW * fb:VAW * fj + VW * (fb + 1)],
                        fex[:, 512:1024],
                        start=(fj == 0), stop=(fj == NLK - 1))

            # ---- attention: q-slice outer; slice s-1's output projection
            # interleaved into slice s's j-loop to keep the exp stream dense
            for si in range(NS):
                cols = slice(512 * si, 512 * (si + 1))
                for p in range(2):
                    hA, hB = 2 * p, 2 * p + 1
                    pa = {0: ps_w.tile([128, 512], F32, tag="mm",
                                       name=f"pa{si}_{p}_0"),
                          1: ps_w2.tile([128, 512], F32, tag="mm2",
                                        name=f"pa{si}_{p}_1")}
                    for j in range(NLK):
                        ks = slice(128 * j, 128 * (j + 1))
                        st = ps_s.tile([128, HALF], F32, tag="s")
                        # concurrent PE row-group pair: head A rows 0:64,
                        # head B rows 64:128, disjoint PSUM banks
                        nc.tensor.matmul(
                            st[:, 0:512], kT[p][0:64, ks], qT[p][0:64, cols],
                            start=True, stop=True)
                        nc.tensor.matmul(
                            st[:, 512:1024], kT[p][64:128, ks],
                            qT[p][64:128, cols], start=True, stop=True)
                        ex = expp.tile([128, HALF], F16, tag="expS")
                        nc.scalar.activation(ex[:], st[:], EXP, scale=SCALE)
                        # attnV runs one iteration behind its exp so the PE
                        # never waits on the exp just issued
                        flush_attnv()
                        pend.append((pa, hA, hB, j, ex))
                        if si > 0 and j in (2, 7, 10, 14):
                            out_proj_group(4 * p + OPS_SLOT[j], si - 1)
                        if si < NS - 1:
                            if p == 0 and j == 0 and (si + 1) not in xt_cache:
                                xt_dma(si + 1)
                            if 3 <= j <= 6:
                                q_proj_chunk(si + 1, p, j - 3)
                    # normalize: attnT = attnU * (1/d); d-block replicated to
                    # 64 partitions via PSUM->SBUF shifts (SBUF->SBUF illegal)
                    flush_attnv()
                    for hh in (1, 0):
                        r0 = 64 * hh
                        pan = pa[hh]
                        dsb = rdp.tile([64, 512], F32, tag="dsb")
                        nc.vector.tensor_copy(dsb[:], pan[64:128, :])
                        rd = rdp.tile([64, 512], F32, tag="rd")
                        rds = rdp.tile([64, 512], F32, tag="rds")
                        if hh == 1:
                            # single-buffered pool: copy out fast to free it
                            scr = scrp.tile([64, 512], F32, tag="scr",
                                            name=f"scr{si}_{p}")
                            nc.vector.tensor_copy(scr[:], pan[0:64, :])
                            src_rows = scr[:]
                        else:
                            src_rows = pan[0:64, :]
                        nc.vector.reciprocal_approx_accurate(
                            rd[:], dsb[:], rds[:])
                        if hh == 1:
                            nc.gpsimd.tensor_mul(
                                aT[p][r0:r0 + 64, cols], src_rows, rd[:])
                        else:
                            nc.vector.tensor_mul(
                                aT[p][r0:r0 + 64, cols], src_rows, rd[:])
            for mo in range(D // 128):
                out_proj_group(mo, NS - 1,
                               pool=[ps_o, ps_w2, ps_w, ps_s][mo % 4],
                               on_act=(mo % 2 == 1))

    nc.compile()
    return nc


_NC_CACHE = []


def _get_nc():
    if not _NC_CACHE:
        _NC_CACHE.append(_build())
    return _NC_CACHE[0]


def kernel_run(inputs, trace=False, **kw):
    """Run on HW; returns (full_output, BassKernelResults)."""
    x = np.asarray(inputs["x"], np.float32)
    context = np.asarray(inputs["context"], np.float32)
    w_q = np.asarray(inputs["w_q"], np.float32)
    b_q = np.asarray(inputs["b_q"], np.float32)
    w_k = np.asarray(inputs["w_k"], np.float32)
    b_k = np.asarray(inputs["b_k"], np.float32)
    w_v = np.asarray(inputs["w_v"], np.float32)
    b_v = np.asarray(inputs["b_v"], np.float32)
    w_o = np.asarray(inputs["w_o"], np.float32)
    b_o = np.asarray(inputs["b_o"], np.float32)

    f16 = np.float16
    xT_h = [np.ascontiguousarray(x[b].T).astype(f16) for b in range(B)]
    cT_h = [np.ascontiguousarray(context[b].T).astype(f16) for b in range(B)]

    maps = []
    for c in range(8):
        b, g = c // 4, c % 4
        hs = slice(256 * g, 256 * (g + 1))
        maps.append({
            "xT": xT_h[b],
            "ctxT": cT_h[b],
            "wq": np.ascontiguousarray(w_q[:, hs]).astype(f16),
            "wk": np.ascontiguousarray(w_k[:, hs]).astype(f16),
            "wv": np.ascontiguousarray(w_v[:, hs]).astype(f16),
            "wo": np.ascontiguousarray(w_o[hs, :]).astype(f16),
            "bq": np.ascontiguousarray(b_q[hs].reshape(2, 128).T),
            "bk": np.ascontiguousarray(b_k[hs].reshape(2, 128).T),
            "bvb": np.ascontiguousarray(
                np.broadcast_to(b_v[None, hs], (128, GD)).astype(np.float32)),
        })

    nc = _get_nc()
    res = bass_utils.run_bass_kernel_spmd(nc, maps, core_ids=list(range(8)),
                                          trace=trace, **kw)
    out = np.empty((B, LQ, D), np.float32)
    for b in range(B):
        acc = res.results[4 * b]["outT"].astype(np.float32)
        for g in range(1, 4):
            acc = acc + res.results[4 * b + g]["outT"].astype(np.float32)
        out[b] = acc.T + b_o[None, :]
    return out, res


def kernel(**inputs) -> np.ndarray:
    out, _ = kernel_run(inputs)
    return out


# revision 39
# speedup vs baseline: 1.1875x; 1.0022x over previous
"""Cross-attention Trainium2 kernel (nn_CrossAttention, B=2, L=2048, D=1024,
Dctx=768, 16 heads x 64).

Sharding: 8 cores = 2 (batch) x 4 (head-groups of 4 heads). Each core computes
its batch's Q/K/V projections for its 4 heads, flash-style attention in the
transposed (S^T) domain, and a partial output projection; the host sums the
head-group partials and adds b_o.

All activations live transposed on-chip (xT, ctxT, qT, kT, attnT) so every
matmul contracts over the partition dim with no on-chip transposes; operands
are fp16 (full PE streaming rate) with fp32 PSUM accumulation. Heads are
processed in pairs: the pair's scores matmuls contract K=64 on PE row-groups
(0,0) and (64,0) and stream CONCURRENTLY into the two banks of one [128,1024]
PSUM tile, so a head-pair's scores cost one stream instead of two. One
1024-wide exp covers both heads. V tiles are padded to 128 columns (64 v + 32
ones for the softmax denominator + 32 zeros) so every stationary load takes
the fast-weight-load path. Output partials are fp16, summed on the host.
"""
import numpy as np

import concourse.bass as bass
import concourse.tile as tile
from concourse import bacc, mybir, bass_utils

F16 = mybir.dt.float16
F32 = mybir.dt.float32
EXP = mybir.ActivationFunctionType.Exp
IDENT = mybir.ActivationFunctionType.Identity

# Problem shape (hardcoded per harness contract)
B, LQ, D = 2, 2048, 1024
DCTX = 768
NH, HD = 16, 64
SCALE = 1.0 / 8.0  # 1/sqrt(64)

# Per-core shard: 4 heads (one group), one batch
GH = 4                # heads per core
ONES = 64             # d-replication rows per head
VW = 128              # per-head v_t width: 64 v + 64 ones (FWL needs 128 cols)
VAW = GH * VW         # 512
GD = GH * HD          # 256: real v columns per chunk
KT_Q = D // 128       # 8
KT_C = DCTX // 128    # 6
NLK = LQ // 128       # 16 key tiles
NS = LQ // 512        # 4 query 512-slices
HALF = 1024
OPS_SLOT = {2: 0, 7: 1, 10: 2, 14: 3}


def _build():
    nc = bacc.Bacc("TRN2", target_bir_lowering=False, debug=False,
                   enable_asserts=False, num_devices=8)

    xT_d = nc.dram_tensor("xT", (D, LQ), F16, kind="ExternalInput").ap()
    cT_d = nc.dram_tensor("ctxT", (DCTX, LQ), F16, kind="ExternalInput").ap()
    wq_d = nc.dram_tensor("wq", (D, 256), F16, kind="ExternalInput").ap()
    wk_d = nc.dram_tensor("wk", (DCTX, 256), F16, kind="ExternalInput").ap()
    wv_d = nc.dram_tensor("wv", (DCTX, GD), F16, kind="ExternalInput").ap()
    wo_d = nc.dram_tensor("wo", (256, D), F16, kind="ExternalInput").ap()
    bq_d = nc.dram_tensor("bq", (128, 2), F32, kind="ExternalInput").ap()
    bk_d = nc.dram_tensor("bk", (128, 2), F32, kind="ExternalInput").ap()
    bvb_d = nc.dram_tensor("bvb", (128, GD), F32, kind="ExternalInput").ap()
    out_d = nc.dram_tensor("outT", (D, LQ), F16, kind="ExternalOutput").ap()

    with tile.TileContext(nc) as tc:
        with tc.tile_pool(name="w", bufs=1) as wp, \
             tc.tile_pool(name="xt", bufs=18) as xtp, \
             tc.tile_pool(name="ct", bufs=24) as ctp, \
             tc.tile_pool(name="act", bufs=1) as actp, \
             tc.tile_pool(name="expp", bufs=6) as expp, \
             tc.tile_pool(name="scrp", bufs=3) as scrp, \
             tc.tile_pool(name="rdp", bufs=4) as rdp, \
             tc.tile_pool(name="outp", bufs=3) as outp, \
             tc.tile_pool(name="ps_w", bufs=2, space="PSUM") as ps_w, \
             tc.tile_pool(name="ps_s", bufs=2, space="PSUM") as ps_s, \
             tc.tile_pool(name="ps_w2", bufs=1, space="PSUM") as ps_w2, \
             tc.tile_pool(name="ps_o", bufs=1, space="PSUM") as ps_o:

            # ---- weight/bias tiles (DMAs issued interleaved below) ----
            wq_t = wp.tile([128, KT_Q * 256], F16, tag="wq")
            wk_t = wp.tile([128, KT_C * 256], F16, tag="wk")
            wv_t = wp.tile([128, KT_C * GD], F16, tag="wv")
            wo_t = wp.tile([128, 2 * D], F16, tag="wo")
            bq_t = wp.tile([128, 2], F32, tag="bq")
            bk_t = wp.tile([128, 2], F32, tag="bk")
            bvb_t = wp.tile([128, GD], F32, tag="bvb")

            # K proj needs these first
            nc.sync.dma_start(wk_t[:].rearrange("p (kt m) -> p kt m", m=256),
                              wk_d.rearrange("(kt p) m -> p kt m", p=128))
            nc.sync.dma_start(bk_t[:], bk_d[:])

            # ---- persistent activation tiles ----
            qT = [actp.tile([128, LQ], F16, tag=f"qT{p}", name=f"qT{p}")
                  for p in range(2)]
            kT = [actp.tile([128, LQ], F16, tag=f"kT{p}", name=f"kT{p}")
                  for p in range(2)]
            v_t = actp.tile([128, NLK * VAW], F16, tag="v")
            aT = [actp.tile([128, LQ], F16, tag=f"aT{p}", name=f"aT{p}")
                  for p in range(2)]

            # constant ones (softmax denominator) / zero-pad rows of v_t
            v4 = v_t[:].rearrange("p (j w) -> p j w", w=VAW)
            for h in range(GH):
                nc.vector.memset(v4[:, :, VW * h + HD:VW * (h + 1)], 1.0)

            # ---- K+V projections interleaved per ctx 512-slice ----
            head_ps_rot = [0]
            ct_tiles = {}
            for s in range(NS):
                for kt in range(KT_C):
                    t = ctp.tile([128, 512], F16, tag="ct")
                    nc.sync.dma_start(
                        t[:], cT_d[128 * kt:128 * (kt + 1), 512 * s:512 * (s + 1)])
                    ct_tiles[(kt, s)] = t
                # stagger the remaining input loads behind the ct slices
                if s == 0:
                    nc.sync.dma_start(
                        wv_t[:].rearrange("p (kt m) -> p kt m", m=GD),
                        wv_d.rearrange("(kt p) m -> p kt m", p=128))
                    nc.sync.dma_start(bvb_t[:], bvb_d[:])
                elif s == 1:
                    nc.sync.dma_start(
                        wq_t[:].rearrange("p (kt m) -> p kt m", m=256),
                        wq_d.rearrange("(kt p) m -> p kt m", p=128))
                    nc.sync.dma_start(bq_t[:], bq_d[:])
                elif s == 2:
                    nc.sync.dma_start(
                        wo_t[:].rearrange("p (p2 m) -> p p2 m", m=1024),
                        wo_d.rearrange("(p2 p) m -> p p2 m", p=128))
                    xt_dma(0)
                def head_ps(name):
                    k = head_ps_rot[0]
                    head_ps_rot[0] = (k + 1) % 3
                    if k == 1:
                        return ps_s.tile([128, HALF], F32, tag="s",
                                         name=name)[:, 0:512]
                    if k == 2:
                        return ps_w2.tile([128, 512], F32, tag="mm2",
                                          name=name)[:]
                    return ps_w.tile([128, 512], F32, tag="mm", name=name)[:]

                for p in range(2):
                    ps = head_ps(f"kps{s}_{p}")
                    for kt in range(KT_C):
                        nc.tensor.matmul(
                            ps, wk_t[:, 256 * kt + 128 * p:256 * kt + 128 * (p + 1)],
                            ct_tiles[(kt, s)][:],
                            start=(kt == 0), stop=(kt == KT_C - 1))
                    nc.scalar.activation(
                        kT[p][:, 512 * s:512 * (s + 1)], ps, IDENT,
                        bias=bk_t[:, p:p + 1])
                for jj in range(4):
                    j = 4 * s + jj
                    ps = head_ps(f"vps{j}")
                    for kt in range(KT_C):
                        nc.tensor.matmul(
                            ps[:, 0:GD],
                            ct_tiles[(kt, s)][:, 128 * jj:128 * (jj + 1)],
                            wv_t[:, GD * kt:GD * (kt + 1)],
                            start=(kt == 0), stop=(kt == KT_C - 1))
                    for h in range(GH):
                        nc.vector.tensor_add(
                            v_t[:, VAW * j + VW * h:VAW * j + VW * h + HD],
                            ps[:, HD * h:HD * (h + 1)],
                            bvb_t[:, HD * h:HD * (h + 1)])

            # ---- Q projection; slice 0 up front, rest interleaved into
            # attention (q-slice si+1 computed during attention on si)
            xt_cache = {}

            def xt_dma(s):
                tiles = []
                for kt in range(KT_Q):
                    t = xtp.tile([128, 512], F16, tag="xt",
                                 name=f"xt{s}_{kt}")
                    nc.sync.dma_start(
                        t[:], xT_d[128 * kt:128 * (kt + 1), 512 * s:512 * (s + 1)])
                    tiles.append(t)
                xt_cache[s] = tiles
                return tiles

            qps_live = {}

            def q_proj_chunk(s, pp, ci):
                xt_tiles = xt_cache.get(s) or xt_dma(s)
                if ci == 0:
                    qps_live[(s, pp)] = ps_o.tile(
                        [128, 512], F32, tag="o", name=f"qps{s}_{pp}")
                ps = qps_live[(s, pp)]
                for kt in range(2 * ci, 2 * ci + 2):
                    nc.tensor.matmul(
                        ps[:], wq_t[:, 256 * kt + 128 * pp:256 * kt + 128 * (pp + 1)],
                        xt_tiles[kt][:],
                        start=(kt == 0), stop=(kt == KT_Q - 1))
                if ci == 3:
                    qsc = scrp.tile([128, 512], F32, tag="scr",
                                    name=f"qsc{s}_{pp}")
                    nc.vector.tensor_scalar_add(qsc[:], ps[:], bq_t[:, pp:pp + 1])
                    nc.vector.tensor_copy(qT[pp][:, 512 * s:512 * (s + 1)], qsc[:])

            def q_proj(s, pp, on_act=False):
                xt_tiles = xt_cache.get(s) or xt_dma(s)
                ps = ps_o.tile([128, 512], F32, tag="o", name=f"qps{s}_{pp}")
                for kt in range(KT_Q):
                    nc.tensor.matmul(
                        ps[:], wq_t[:, 256 * kt + 128 * pp:256 * kt + 128 * (pp + 1)],
                        xt_tiles[kt][:],
                        start=(kt == 0), stop=(kt == KT_Q - 1))
                if on_act:
                    nc.scalar.activation(
                        qT[pp][:, 512 * s:512 * (s + 1)], ps[:], IDENT,
                        bias=bq_t[:, pp:pp + 1])
                else:
                    qsc = scrp.tile([128, 512], F32, tag="scr",
                                    name=f"qsc{s}_{pp}")
                    nc.vector.tensor_scalar_add(qsc[:], ps[:], bq_t[:, pp:pp + 1])
                    nc.vector.tensor_copy(qT[pp][:, 512 * s:512 * (s + 1)], qsc[:])

            for pp in range(2):
                q_proj(0, pp, on_act=True)

            def out_proj_group(mo, s, pool=None, on_act=False):
                pool = pool if pool is not None else ps_o
                if pool is ps_s:
                    ops = ps_s.tile([128, HALF], F32, tag="s",
                                    name=f"ops{mo}_{s}")[:, 0:512]
                else:
                    tg = "o" if pool is ps_o else ("mm" if pool is ps_w else "mm2")
                    ops = pool.tile([128, 512], F32, tag=tg, name=f"ops{mo}_{s}")
                for p in range(2):
                    nc.tensor.matmul(
                        ops, wo_t[:, D * p + 128 * mo:D * p + 128 * (mo + 1)],
                        aT[p][:, 512 * s:512 * (s + 1)],
                        start=(p == 0), stop=(p == 1))
                ot = outp.tile([128, 512], F16, tag="out")
                if on_act:
                    nc.scalar.copy(ot[:], ops)
                else:
                    nc.vector.tensor_copy(ot[:], ops)
                nc.sync.dma_start(
                    out_d[128 * mo:128 * (mo + 1), 512 * s:512 * (s + 1)], ot[:])

            pend = []

            def flush_attnv():
                while pend:
                    fpa, fa, fb, fj, fex = pend.pop()
                    nc.tensor.matmul(
                        fpa[0][:],
                        v_t[:, VAW * fj + VW * fa:VAW * fj + VW * (fa + 1)],
                        fex[:, 0:512],
                        start=(fj == 0), stop=(fj == NLK - 1))
                    nc.tensor.matmul(
                        fpa[1][:],
                        v_t[:, VAW * fj + VW * fb:VAW * fj + VW * (fb + 1)],
                        fex[:, 512:1024],
                        start=(fj == 0), stop=(fj == NLK - 1))

            # ---- attention: q-slice outer; slice s-1's output projection
            # interleaved into slice s's j-loop to keep the exp stream dense
            for si in range(NS):
                cols = slice(512 * si, 512 * (si + 1))
                for p in range(2):
                    hA, hB = 2 * p, 2 * p + 1
                    pa = {0: ps_w.tile([128, 512], F32, tag="mm",
                                       name=f"pa{si}_{p}_0"),
                          1: ps_w2.tile([128, 512], F32, tag="mm2",
                                        name=f"pa{si}_{p}_1")}
                    for j in range(NLK):
                        ks = slice(128 * j, 128 * (j + 1))
                        st = ps_s.tile([128, HALF], F32, tag="s")
                        # concurrent PE row-group pair: head A rows 0:64,
                        # head B rows 64:128, disjoint PSUM banks
                        nc.tensor.matmul(
                            st[:, 0:512], kT[p][0:64, ks], qT[p][0:64, cols],
                            start=True, stop=True)
                        nc.tensor.matmul(
                            st[:, 512:1024], kT[p][64:128, ks],
                            qT[p][64:128, cols], start=True, stop=True)
                        ex = expp.tile([128, HALF], F16, tag="expS")
                        nc.scalar.activation(ex[:], st[:], EXP, scale=SCALE)
                        # attnV runs one iteration behind its exp so the PE
                        # never waits on the exp just issued
                        flush_attnv()
                        pend.append((pa, hA, hB, j, ex))
                        if si > 0 and j in (2, 7, 10, 14):
                            out_proj_group(4 * p + OPS_SLOT[j], si - 1)
                        if si < NS - 1:
                            if p == 0 and j == 0 and (si + 1) not in xt_cache:
                                xt_dma(si + 1)
                            if 3 <= j <= 6:
                                q_proj_chunk(si + 1, p, j - 3)
                    # normalize: attnT = attnU * (1/d); d-block replicated to
                    # 64 partitions via PSUM->SBUF shifts (SBUF->SBUF illegal)
                    flush_attnv()
                    for hh in (1, 0):
                        r0 = 64 * hh
                        pan = pa[hh]
                        dsb = rdp.tile([64, 512], F32, tag="dsb")
                        nc.vector.tensor_copy(dsb[:], pan[64:128, :])
                        rd = rdp.tile([64, 512], F32, tag="rd")
                        rds = rdp.tile([64, 512], F32, tag="rds")
                        if hh == 1:
                            # single-buffered pool: copy out fast to free it
                            scr = scrp.tile([64, 512], F32, tag="scr",
                                            name=f"scr{si}_{p}")
                            nc.vector.tensor_copy(scr[:], pan[0:64, :])
                            src_rows = scr[:]
                        else:
                            src_rows = pan[0:64, :]
                        nc.vector.reciprocal_approx_accurate(
                            rd[:], dsb[:], rds[:])
                        if hh == 1:
                            nc.gpsimd.tensor_mul(
                                aT[p][r0:r0 + 64, cols], src_rows, rd[:])
                        else:
                            nc.vector.tensor_mul(
                                aT[p][r0:r0 + 64, cols], src_rows, rd[:])
            for mo in range(D // 128):
                out_proj_group(mo, NS - 1,
                               pool=[ps_o, ps_w2, ps_w, ps_s][mo % 4],
                               on_act=(mo % 2 == 1))

    nc.compile()
    return nc


_NC_CACHE = []


def _get_nc():
    if not _NC_CACHE:
        _NC_CACHE.append(_build())
    return _NC_CACHE[0]


def kernel_run(inputs, trace=False, **kw):
    """Run on HW; returns (full_output, BassKernelResults)."""
    x = np.asarray(inputs["x"], np.float32)
    context = np.asarray(inputs["context"], np.float32)
    w_q = np.asarray(inputs["w_q"], np.float32)
    b_q = np.asarray(inputs["b_q"], np.float32)
    w_k = np.asarray(inputs["w_k"], np.float32)
    b_k = np.asarray(inputs["b_k"], np.float32)
    w_v = np.asarray(inputs["w_v"], np.float32)
    b_v = np.asarray(inputs["b_v"], np.float32)
    w_o = np.asarray(inputs["w_o"], np.float32)
    b_o = np.asarray(inputs["b_o"], np.float32)

    f16 = np.float16
    xT_h = [np.ascontiguousarray(x[b].T).astype(f16) for b in range(B)]
    cT_h = [np.ascontiguousarray(context[b].T).astype(f16) for b in range(B)]

    maps = []
    for c in range(8):
        b, g = c // 4, c % 4
        hs = slice(256 * g, 256 * (g + 1))
        maps.append({
            "xT": xT_h[b],
            "ctxT": cT_h[b],
            "wq": np.ascontiguousarray(w_q[:, hs]).astype(f16),
            "wk": np.ascontiguousarray(w_k[:, hs]).astype(f16),
            "wv": np.ascontiguousarray(w_v[:, hs]).astype(f16),
            "wo": np.ascontiguousarray(w_o[hs, :]).astype(f16),
            "bq": np.ascontiguousarray(b_q[hs].reshape(2, 128).T),
            "bk": np.ascontiguousarray(b_k[hs].reshape(2, 128).T),
            "bvb": np.ascontiguousarray(
                np.broadcast_to(b_v[None, hs], (128, GD)).astype(np.float32)),
        })

    nc = _get_nc()
    res = bass_utils.run_bass_kernel_spmd(nc, maps, core_ids=list(range(8)),
                                          trace=trace, **kw)
    out = np.empty((B, LQ, D), np.float32)
    for b in range(B):
        acc = res.results[4 * b]["outT"].astype(np.float32)
        for g in range(1, 4):
            acc = acc + res.results[4 * b + g]["outT"].astype(np.float32)
        out[b] = acc.T + b_o[None, :]
    return out, res


def kernel(**inputs) -> np.ndarray:
    out, _ = kernel_run(inputs)
    return out
